# revision 34
# baseline (speedup 1.0000x reference)
"""LeViT-style attention block (qkv+BN -> biased softmax attention -> hardswish -> proj+BN)
on 8 Trainium2 NeuronCores, data-parallel over the batch dimension.

Self-contained: hardcodes shapes B=16, N=784, C=384, H=8.

Key device-side structure (per core, 2 batches):
- qkv matmul in bf16 (fp32 PSUM accumulate), channel slots padded to 64 per head.
  PSUM evacuation runs on the otherwise-idle ScalarE (ACT copy) so VectorE is
  free for the BN1 stats chain.
- BN1 statistics: Gram matrix G = [x|1]^T [x|1] on the PE concurrently with the
  qkv matmul, then H = W G locally and per-slot (sum y^2, sum y) packed into a
  12KB fp32 AllReduce (instead of AllReducing the 295KB Gram matrix).  1/sigma
  via exp(-0.5 ln(var+eps)) so the ACT table never swaps away from ln/exp.
- attention per (head-pair, batch): scores accumulate in PSUM, exp on ScalarE
  straight from PSUM, per-chunk multiply by the gathered exp(rel-pos bias) on
  VectorE/GpSimd, ones-row folded into V gives the softmax denominator,
  denominator reciprocal via exp(-ln d) on ScalarE + DMA partition-broadcast.
  The PE is power-throttled to ~50% duty after the first ~40us, so the design
  minimizes PE rows and keeps elementwise work off the PE.
- hardswish on VectorE; proj matmul + BN2 (bn_stats + small AllReduce).
"""
import os
import numpy as np
import ml_dtypes

import concourse.bass as bass
import concourse.mybir as mybir
import concourse.tile as tile
from concourse import bacc
from concourse.bass_utils import run_bass_kernel_spmd

F32 = mybir.dt.float32
BF16 = mybir.dt.bfloat16
BF = ml_dtypes.bfloat16

B, N, C, H = 16, 784, 384, 8
HD = 48            # head dim
HP = 64            # padded head dim (slot block size)
NCORES = 8
NB = B // NCORES   # batches per core = 2
R = NB * N         # rows per core = 1568
S1 = 3 * H * HP    # qkv slot count = 1536
MC1 = S1 // 128    # qkv m-chunks = 12
S2 = H * HP        # proj input slots = 512
KC = C // 128      # input channel chunks = 3
RC = 13            # row chunks for the Gram matmul (1568 -> 13*128 = 1664 padded)
EPS = 1e-5
SCALE = HD ** -0.5
NTOT = B * N       # total rows for BN stats = 12544
MCH = 7            # attention m-chunks (of 112) per 784
MCS = 112
# free-dim chunks that respect the 512-element PSUM bank boundary
NCH = [(0, 512), (512, 272)]
NCH_R = [(0, 512), (512, 512), (1024, 512), (1536, 32)]

_CACHE = {}

# Restrict the ACT table-set choices so exp, ln and copy share one set
# (natural_log_exp_and_others); the kernel only uses those so no
# ACT_TABLE_LOAD ever fires mid-kernel.
_orig_get_tables = bacc.get_activation_tables


def _patched_get_tables(arch):
    # Preserve entry order (act_func_set_id = index into act_info.json) but
    # empty the function lists of non-preferred sets so the chooser can't
    # pick them.
    t = _orig_get_tables(arch)
    keep = {"natural_log_exp_and_others"}
    return {k: (v if k in keep else set()) for k, v in t.items()}


bacc.get_activation_tables = _patched_get_tables


def _build():
    nc = bacc.Bacc("TRN2", target_bir_lowering=False, debug=False, num_devices=NCORES)

    xT_d = nc.dram_tensor("xT", [128, KC, R], BF16, kind="ExternalInput")
    xN_d = nc.dram_tensor("xN", [128, RC, C + 1], BF16, kind="ExternalInput")
    wq_d = nc.dram_tensor("wqT", [128, KC, S1], BF16, kind="ExternalInput")
    wqn_d = nc.dram_tensor("wqN", [128, MC1, C], BF16, kind="ExternalInput")
    wp_d = nc.dram_tensor("wpT", [128, S2 // 128, C], BF16, kind="ExternalInput")
    bias_d = nc.dram_tensor("biasg", [H, MCH, MCS, N], BF16, kind="ExternalInput")
    g1_d = nc.dram_tensor("g1p", [128, MC1], F32, kind="ExternalInput")
    b1_d = nc.dram_tensor("b1p", [128, MC1], F32, kind="ExternalInput")
    g2_d = nc.dram_tensor("g2p", [128, KC], F32, kind="ExternalInput")
    b2_d = nc.dram_tensor("b2p", [128, KC], F32, kind="ExternalInput")
    id_d = nc.dram_tensor("idm", [128, 128], BF16, kind="ExternalInput")
    ids_d = nc.dram_tensor("ids", [HP, HP], BF16, kind="ExternalInput")
    out_d = nc.dram_tensor("outT", [128, KC, R], F32, kind="ExternalOutput")

    cc1_in = nc.dram_tensor("cc1_in", [128, MC1, 2], F32)
    cc1_out = nc.dram_tensor("cc1_out", [128, MC1, 2], F32, addr_space="Shared")
    rscr_d = nc.dram_tensor("rscr", [H // 2, NB, 2, N], F32)
    cc2_in = nc.dram_tensor("cc2_in", [128, KC, 2], F32)
    cc2_out = nc.dram_tensor("cc2_out", [128, KC, 2], F32, addr_space="Shared")

    AF = mybir.ActivationFunctionType
    OP = mybir.AluOpType
    PH = int(os.environ.get("BASS_ATT_PHASES", "4"))

    with tile.TileContext(nc) as tc:
        with tc.tile_pool(name="singles", bufs=1) as singles:
            xNs = singles.tile([128, RC, C + 1], BF16, tag="xN_s")
            nc.sync.dma_start(out=xNs[:], in_=xN_d.ap())
            wq = singles.tile([128, KC, S1], BF16)
            nc.sync.dma_start(out=wq[:], in_=wq_d.ap())
            wp = singles.tile([128, S2 // 128, C], BF16)
            nc.sync.dma_start(out=wp[:], in_=wp_d.ap())
            idm = singles.tile([128, 128], BF16)
            nc.sync.dma_start(out=idm[:], in_=id_d.ap())
            ids = singles.tile([HP, HP], BF16)
            nc.sync.dma_start(out=ids[:], in_=ids_d.ap())
            g1 = singles.tile([128, MC1], F32)
            nc.sync.dma_start(out=g1[:], in_=g1_d.ap())
            b1 = singles.tile([128, MC1], F32)
            nc.sync.dma_start(out=b1[:], in_=b1_d.ap())
            g2 = singles.tile([128, KC], F32)
            nc.sync.dma_start(out=g2[:], in_=g2_d.ap())
            b2 = singles.tile([128, KC], F32)
            nc.sync.dma_start(out=b2[:], in_=b2_d.ap())
            eps_t = singles.tile([128, 1], F32)
            nc.vector.memset(eps_t[:], EPS)

            y = singles.tile([128, MC1, R], BF16)            # qkv out, normalized in place
            o_pad = singles.tile([128, S2 // 128, R], BF16)  # attention output (padded slots)
            y2 = singles.tile([128, KC, R], F32)             # proj out, normalized in place
            scale1 = singles.tile([128, MC1], F32)
            shift1 = singles.tile([128, MC1], F32)
            lnd = singles.tile([33, N], F32)
            nc.vector.memset(lnd[:], 0.0)

            # ---------------- Phase 1: Gram stats (PE) + local H + small AllReduce, qkv matmul ----------------
            with tc.tile_pool(name="ph1", bufs=1) as ph1, \
                 tc.tile_pool(name="psg", bufs=2, space="PSUM") as psg, \
                 tc.tile_pool(name="psq", bufs=2, space="PSUM") as psq:
                xN = xNs
                xT = ph1.tile([128, KC, R], BF16)
                nc.sync.dma_start(out=xT[:], in_=xT_d.ap())
                wqn = ph1.tile([128, MC1, C], BF16)
                nc.sync.dma_start(out=wqn[:], in_=wqn_d.ap())
                gram_sb = ph1.tile([128, KC, C + 1], BF16)

                # Gram matrix G = [x|1]^T [x|1]  (col C holds the row-sums of x)
                for mc in range(KC):
                    gps = psg.tile([128, C + 1], F32, tag="gram")
                    for rc in range(RC):
                        nc.tensor.matmul(
                            gps[:],
                            xN[:, rc, mc * 128:(mc + 1) * 128],
                            xN[:, rc, :],
                            start=(rc == 0), stop=(rc == RC - 1),
                        )
                    nc.scalar.activation(gram_sb[:, mc, :], gps[:], AF.Copy)

                # H = W_pad @ G_local: per slot-chunk, H[:, :C] for sum(y^2),
                # H[:, C] = local sum(y).  Pack (sum y^2, sum y) per slot and
                # AllReduce just that (12KB fp32) instead of the Gram matrix.
                sumsq = ph1.tile([128, MC1], F32)
                sumY = ph1.tile([128, MC1], F32)
                scratch = ph1.tile([128, 2, C], F32)
                hsb = ph1.tile([128, MC1, C + 1], F32)
                for mc in range(MC1):
                    hps = psg.tile([128, C + 1], F32, tag="hmat")
                    for kc in range(KC):
                        nc.tensor.matmul(
                            hps[:],
                            wq[:, kc, mc * 128:(mc + 1) * 128],
                            gram_sb[:, kc, :],
                            start=(kc == 0), stop=(kc == KC - 1),
                        )
                    # fast PSUM evacuation on ScalarE so the slower VectorE
                    # reduction chain below doesn't pace the PE via the pool
                    nc.scalar.activation(hsb[:, mc, :], hps[:], AF.Copy)
                for mc in range(MC1):
                    # sumsq[slot] = sum_k W[slot,k] * H[slot,k]
                    sc = scratch[:, mc % 2, :]
                    nc.vector.tensor_tensor(sc, wqn[:, mc, :], hsb[:, mc, 0:C], op=OP.mult)
                    nc.vector.reduce_sum(sumsq[:, mc:mc + 1], sc, axis=mybir.AxisListType.X)
                    nc.vector.tensor_copy(sumY[:, mc:mc + 1], hsb[:, mc, C:C + 1])

                cc1_sb = ph1.tile([128, MC1, 2], F32)
                nc.vector.tensor_copy(cc1_sb[:, :, 0], sumsq[:])
                nc.vector.tensor_copy(cc1_sb[:, :, 1], sumY[:])
                cc1g = singles.tile([128, MC1, 2], F32)
                nc.gpsimd.dma_start(out=cc1_in.ap(), in_=cc1_sb[:])
                nc.gpsimd.collective_compute(
                    "AllReduce", OP.add,
                    ins=[cc1_in.ap()], outs=[cc1_out.ap()],
                    replica_groups=[list(range(NCORES))],
                )


                # qkv matmul (overlaps the Gram/AllReduce chain above); the
                # PSUM evacuation runs on ScalarE (ACT copy w/ cast) so the
                # VectorE stats chain above is never queued behind it.
                MC_ORDER = [0, 4, 8, 1, 5, 9, 2, 6, 10, 3, 7, 11]
                for mc in MC_ORDER:
                    for half in range(2):
                        ps = psq.tile([128, N], F32, tag="ps1")
                        h0 = half * N
                        for kc in range(KC):
                            for (off, sz) in NCH:
                                nc.tensor.matmul(
                                    ps[:, off:off + sz],
                                    wq[:, kc, mc * 128:(mc + 1) * 128],
                                    xT[:, kc, h0 + off:h0 + off + sz],
                                    start=(kc == 0), stop=(kc == KC - 1),
                                )
                        nc.scalar.activation(y[:, mc, h0:h0 + N], ps[:], AF.Copy)

            # ---------------- Phase 2a: v transposes on RAW v (no stats dep) ----------------
            # v_norm = a*v_raw + b and attention rows sum to 1 after the
            # denominator divide, so o = a*(attn@v_raw) + b: the BN affine is
            # applied per-partition in the tail instead of normalizing v.  The
            # transposes therefore run in the PE window that previously idled
            # waiting for the BN1 AllReduce (and mostly before the power
            # throttle kicks in); the ones-slot for the softmax denominator is
            # memset directly into the transposed tiles.
            v_exts = {}
            with tc.tile_pool(name="vop", bufs=2) as vop, \
                 tc.tile_pool(name="psvx", bufs=2, space="PSUM") as psvx:
                for h in range(H):
                    hp, eo = h // 2, h % 2
                    for b in range(NB):
                        col0 = b * N
                        # PE transposes need a base-0 source partition: odd
                        # head's v block staged down via DMA first
                        if eo == 0:
                            vs = lambda a, z: y[0:HP, 8 + hp, a:z]
                        else:
                            vodd = vop.tile([HP, N], BF16, tag="vodd")
                            nc.gpsimd.dma_start(
                                out=vodd[:], in_=y[HP:128, 8 + hp, col0:col0 + N])
                            vs = lambda a, z: vodd[:, a - col0:z - col0]
                        psv = psvx.tile([MCS, MCH, HP], BF16, tag="psv")
                        for mc in range(MCH):
                            nc.tensor.transpose(
                                psv[:, mc, :],
                                vs(col0 + mc * MCS, col0 + (mc + 1) * MCS),
                                ids[:],
                            )
                        vx = singles.tile([MCS, MCH, HP], BF16, tag=f"vx{h}_{b}")
                        # evacuate on ScalarE / memset on GpSimd: keeps these
                        # 32 ops off the VectorE queue so the AllReduce-gated
                        # normalize (and with it the first scores/exp) isn't
                        # serialized behind them
                        nc.scalar.activation(vx[:], psv[:], AF.Copy)
                        nc.gpsimd.memset(vx[:, :, 0], 1.0)
                        v_exts[(h, b)] = vx

            # scale/shift from global stats (attn scale folded into g1 on host).
            # This chain (and its ScalarE ln/exp, and the AllReduce-result DMA
            # on the VectorE queue) sits AFTER the raw-v work in program order
            # so no raw-y consumer inherits a semaphore target or in-order
            # queue position behind the AllReduce.
            nc.scalar.dma_start(out=cc1g[:], in_=cc1_out.ap())
            meanY = singles.tile([128, MC1], F32)
            varG = singles.tile([128, MC1], F32)
            nc.vector.tensor_scalar_mul(meanY[:], cc1g[:, :, 1], 1.0 / NTOT)
            nc.vector.tensor_scalar_mul(varG[:], cc1g[:, :, 0], 1.0 / NTOT)
            nc.vector.tensor_tensor(scale1[:], meanY[:], meanY[:], op=OP.mult)
            nc.vector.tensor_tensor(varG[:], varG[:], scale1[:], op=OP.subtract)
            # 1/sigma = exp(-0.5 * ln(var + eps)) -- stays in the ln/exp ACT table set
            nc.scalar.activation(varG[:], varG[:], AF.Ln, bias=eps_t[:])
            nc.scalar.activation(varG[:], varG[:], AF.Exp, scale=-0.5)
            nc.vector.tensor_tensor(scale1[:], g1[:], varG[:], op=OP.mult)
            nc.vector.tensor_tensor(shift1[:], meanY[:], scale1[:], op=OP.mult)
            nc.vector.tensor_tensor(shift1[:], b1[:], shift1[:], op=OP.subtract)

            # normalize q/k chunks in place (v stays raw), ordered so head 0's
            # chunks (q=0, k=4) finish first and attention can start early
            for mc in [0, 4, 1, 5, 2, 6, 3, 7]:
                nc.vector.tensor_scalar(
                    y[:, mc, :], y[:, mc, :],
                    scale1[:, mc:mc + 1], shift1[:, mc:mc + 1],
                    op0=OP.mult, op1=OP.add,
                )

            # ---------------- Phase 3: attention, one head per iteration ----------------
            # Software-pipelined: scores(mc+1) on the PE overlap exp(mc) on the
            # ScalarE (scores psum double-buffered); AV(mc-1) is emitted after
            # scores(mc) so the PE stream never stalls on the exp.
            if PH < 3:
                nc.vector.memset(o_pad[:], 0.0)
            if PH >= 3:
             with tc.tile_pool(name="biasp", bufs=2) as biasp, \
                  tc.tile_pool(name="ep", bufs=3) as ep, \
                  tc.tile_pool(name="tailp", bufs=2) as tailp, \
                  tc.tile_pool(name="pss", bufs=2, space="PSUM") as pss, \
                  tc.tile_pool(name="pso", bufs=2, space="PSUM") as pso:
                def emit_tail(hp, b, eo, lo, o_raw):
                    # r = exp(-ln(denom)); broadcast via DRAM DMA; normalize,
                    # v's BN affine (per-partition), and hardswish on VectorE
                    # into o_pad rows lo:lo+64
                    col0 = b * N
                    nc.scalar.activation(lnd[0:1, :], o_raw[0:1, :], AF.Ln)
                    r2 = tailp.tile([1, N], F32, tag="r2")
                    nc.scalar.activation(r2[:], lnd[0:1, :], AF.Exp, scale=-1.0)
                    rb = tailp.tile([HP, N], F32, tag="rb")
                    rsc = rscr_d.ap()[hp, b]
                    nc.gpsimd.dma_start(out=rsc[eo], in_=r2[0:1, :])
                    nc.gpsimd.dma_start(
                        out=rb[:],
                        in_=bass.AP(tensor=rsc.tensor, offset=rsc.offset + eo * N,
                                    ap=[[0, HP], [1, N]]),
                    )
                    oh = tailp.tile([HP, N], BF16, tag="oh")
                    nc.vector.tensor_tensor(oh[:], o_raw[:], rb[:], op=OP.mult)
                    oh2 = tailp.tile([HP, N], BF16, tag="oh2")
                    nc.vector.tensor_scalar(
                        oh2[:], oh[:],
                        scale1[lo:lo + HP, 8 + hp:8 + hp + 1],
                        shift1[lo:lo + HP, 8 + hp:8 + hp + 1],
                        op0=OP.mult, op1=OP.add,
                    )
                    t1 = tailp.tile([HP, N], BF16, tag="t1")
                    nc.vector.tensor_scalar(t1[:], oh2[:], 1.0 / 6.0, 0.5, op0=OP.mult, op1=OP.add)
                    t2 = tailp.tile([HP, N], BF16, tag="t2")
                    nc.vector.tensor_scalar(t2[:], t1[:], 1.0, 0.0, op0=OP.min, op1=OP.max)
                    nc.vector.tensor_tensor(
                        o_pad[lo:lo + HP, hp, col0:col0 + N], oh2[:], t2[:], op=OP.mult,
                    )

                pending_tail = None
                for h in range(H):
                    hp, eo = h // 2, h % 2
                    lo = eo * HP
                    bias_h = biasp.tile([MCS, MCH, N], BF16, tag="bias")
                    nc.sync.dma_start(
                        out=bias_h[:],
                        in_=bias_d.ap()[h].rearrange("m p n -> p m n"),
                    )
                    for b in range(NB):
                        col0 = b * N
                        # previous iteration's tail goes here so its Ln doesn't
                        # head-of-line block this iteration's exps on ScalarE
                        if pending_tail is not None:
                            emit_tail(*pending_tail)
                            pending_tail = None
                        v_ext = v_exts[(h, b)]

                        po = pso.tile([HP, N], F32, tag="po")
                        es = []
                        for mc in range(MCH):
                            m0 = col0 + mc * MCS
                            ps_s = pss.tile([MCS, N], F32, tag="se")
                            for (off, sz) in NCH:
                                nc.tensor.matmul(
                                    ps_s[:, off:off + sz],
                                    y[lo:lo + HD, 4 + hp, m0:m0 + MCS],
                                    y[lo:lo + HD, hp, col0 + off:col0 + off + sz],
                                    start=True, stop=True, skip_group_check=True,
                                )
                            # emit previous chunk's AV now so the PE stream has
                            # work while this chunk's exp runs on ScalarE
                            if mc >= 1:
                                e_prev, vslot = es[-1]
                                for (off, sz) in NCH:
                                    nc.tensor.matmul(
                                        po[:, off:off + sz], v_ext[:, vslot, :],
                                        e_prev[:, off:off + sz],
                                        start=(vslot == 0), stop=False, skip_group_check=True,
                                    )
                            e_c = ep.tile([MCS, N], BF16, tag="E")
                            nc.scalar.activation(e_c[:], ps_s[:], AF.Exp)
                            em = ep.tile([MCS, N], BF16, tag="Em")
                            nc.vector.tensor_tensor(em[:], e_c[:], bias_h[:, mc, :], op=OP.mult)
                            es.append((em, mc))
                        e_prev, vslot = es[-1]
                        for (off, sz) in NCH:
                            nc.tensor.matmul(
                                po[:, off:off + sz], v_ext[:, vslot, :],
                                e_prev[:, off:off + sz],
                                start=False, stop=True, skip_group_check=True,
                            )
                        # evacuate po early (frees the PSUM slot) and defer the
                        # rest of the tail into the next iteration
                        o_raw = tailp.tile([HP, N], F32, tag="oraw")
                        nc.vector.tensor_copy(o_raw[:], po[:])
                        pending_tail = (hp, b, eo, lo, o_raw)
                if pending_tail is not None:
                    emit_tail(*pending_tail)
                    pending_tail = None

            # ---------------- Phase 4: output projection + BN2 ----------------
            stats2 = singles.tile([128, KC, 4, 6], F32)
            mv2 = singles.tile([128, KC, 2], F32)
            cc2_sb = singles.tile([128, KC, 2], F32)
            cc2g = singles.tile([128, KC, 2], F32)
            with tc.tile_pool(name="psq2", bufs=2, space="PSUM") as psq2:
                for mc in range(KC):
                    ps = psq2.tile([128, R], F32, tag="ps2")
                    for kc in range(S2 // 128):
                        for (off, sz) in NCH_R:
                            nc.tensor.matmul(
                                ps[:, off:off + sz],
                                wp[:, kc, mc * 128:(mc + 1) * 128],
                                o_pad[:, kc, off:off + sz],
                                start=(kc == 0), stop=(kc == S2 // 128 - 1),
                            )
                    nc.vector.tensor_copy(y2[:, mc, :], ps[:])
                    for c4 in range(4):
                        nc.vector.bn_stats(stats2[:, mc, c4, :], y2[:, mc, c4 * 392:(c4 + 1) * 392])
                    nc.vector.bn_aggr(mv2[:, mc, :], stats2[:, mc, :, :])

            nc.vector.tensor_scalar_mul(cc2_sb[:, :, 0], mv2[:, :, 0], float(R))
            nc.vector.tensor_tensor(cc2_sb[:, :, 1], mv2[:, :, 0], mv2[:, :, 0], op=OP.mult)
            nc.vector.tensor_tensor(cc2_sb[:, :, 1], cc2_sb[:, :, 1], mv2[:, :, 1], op=OP.add)
            nc.vector.tensor_scalar_mul(cc2_sb[:, :, 1], cc2_sb[:, :, 1], float(R))
            nc.gpsimd.dma_start(out=cc2_in.ap(), in_=cc2_sb[:])
            nc.gpsimd.collective_compute(
                "AllReduce", OP.add,
                ins=[cc2_in.ap()], outs=[cc2_out.ap()],
                replica_groups=[list(range(NCORES))],
            )
            nc.gpsimd.dma_start(out=cc2g[:], in_=cc2_out.ap())

            mean2 = singles.tile([128, KC], F32)
            var2 = singles.tile([128, KC], F32)
            scale2 = singles.tile([128, KC], F32)
            shift2 = singles.tile([128, KC], F32)
            nc.vector.tensor_scalar_mul(mean2[:], cc2g[:, :, 0], 1.0 / NTOT)
            nc.vector.tensor_scalar_mul(var2[:], cc2g[:, :, 1], 1.0 / NTOT)
            nc.vector.tensor_tensor(scale2[:], mean2[:], mean2[:], op=OP.mult)
            nc.vector.tensor_tensor(var2[:], var2[:], scale2[:], op=OP.subtract)
            nc.scalar.activation(var2[:], var2[:], AF.Ln, bias=eps_t[:])
            nc.scalar.activation(var2[:], var2[:], AF.Exp, scale=-0.5)
            nc.vector.tensor_tensor(scale2[:], g2[:], var2[:], op=OP.mult)
            nc.vector.tensor_tensor(shift2[:], mean2[:], scale2[:], op=OP.mult)
            nc.vector.tensor_tensor(shift2[:], b2[:], shift2[:], op=OP.subtract)
            for mc in range(KC):
                nc.vector.tensor_scalar(
                    y2[:, mc, :], y2[:, mc, :],
                    scale2[:, mc:mc + 1], shift2[:, mc:mc + 1],
                    op0=OP.mult, op1=OP.add,
                )
                nc.sync.dma_start(out=out_d.ap()[:, mc], in_=y2[:, mc, :])

    nc.compile()
    return nc


def _host_prep(x, Wqkv, g1, b1, Wproj, g2, b2, biases, bias_idxs):
    x = np.asarray(x, dtype=np.float32)
    Wqkv = np.asarray(Wqkv, dtype=np.float32)
    g1 = np.asarray(g1, dtype=np.float32)
    b1 = np.asarray(b1, dtype=np.float32)
    Wproj = np.asarray(Wproj, dtype=np.float32)
    g2 = np.asarray(g2, dtype=np.float32)
    b2 = np.asarray(b2, dtype=np.float32)
    biases = np.asarray(biases, dtype=np.float32)
    bias_idxs = np.asarray(bias_idxs)

    # channel slot layout: block = t*8+h (t in q,k,v), 64 slots per block.
    # q/k blocks: dims at slots 0..47.  v blocks: ones-row at slot 0 (32-aligned
    # partition for the denominator), dims at slots 1..48.
    g1e = g1.copy()
    b1e = b1.copy()
    g1e[:C] *= SCALE   # fold attention scale into q channels
    b1e[:C] *= SCALE
    Wq_pad = np.zeros((S1, C), np.float32)
    g1_pad = np.zeros(S1, np.float32)
    b1_pad = np.zeros(S1, np.float32)
    for t in range(3):
        for h in range(H):
            blk = (t * H + h) * HP
            d0 = blk + (1 if t == 2 else 0)
            Wq_pad[d0:d0 + HD] = Wqkv[t * C + h * HD: t * C + h * HD + HD]
            g1_pad[d0:d0 + HD] = g1e[t * C + h * HD: t * C + h * HD + HD]
            b1_pad[d0:d0 + HD] = b1e[t * C + h * HD: t * C + h * HD + HD]
    for h in range(H):
        b1_pad[(2 * H + h) * HP] = 1.0

    wq_host = np.ascontiguousarray(
        Wq_pad.T.reshape(KC, 128, S1).transpose(1, 0, 2)).astype(BF)
    wqn_host = np.ascontiguousarray(
        Wq_pad.reshape(MC1, 128, C).transpose(1, 0, 2)).astype(BF)
    g1_host = np.ascontiguousarray(g1_pad.reshape(MC1, 128).T)
    b1_host = np.ascontiguousarray(b1_pad.reshape(MC1, 128).T)

    Wp_pad = np.zeros((S2, C), np.float32)   # [slot_in, c_out]; slot 0 (denom) stays zero
    for h in range(H):
        Wp_pad[h * HP + 1:h * HP + 1 + HD] = Wproj[:, h * HD:h * HD + HD].T
    wp_host = np.ascontiguousarray(
        Wp_pad.reshape(S2 // 128, 128, C).transpose(1, 0, 2)).astype(BF)
    g2_host = np.ascontiguousarray(g2.reshape(KC, 128).T)
    b2_host = np.ascontiguousarray(b2.reshape(KC, 128).T)

    bias_gather = biases[:, bias_idxs]                     # [H, N, N] fp32
    bias_host = np.ascontiguousarray(
        np.exp(bias_gather).reshape(H, MCH, MCS, N)).astype(BF)

    id_host = np.zeros((128, 128), np.float32)
    id_host[0:112, 0:112] = np.eye(112)
    id_host = id_host.astype(BF)
    ids_host = np.eye(HP, dtype=np.float32).astype(BF)

    shared = {
        "wqT": wq_host, "wqN": wqn_host, "wpT": wp_host, "biasg": bias_host,
        "g1p": g1_host, "b1p": b1_host, "g2p": g2_host, "b2p": b2_host,
        "idm": id_host, "ids": ids_host,
    }
    in_maps = []
    for c in range(NCORES):
        xc = x[NB * c: NB * (c + 1)].reshape(R, C)         # [R, C]
        xcT = np.ascontiguousarray(
            xc.T.reshape(KC, 128, R).transpose(1, 0, 2)).astype(BF)
        # natural layout rows padded to RC*128 with an appended ones column
        xn = np.zeros((RC * 128, C + 1), np.float32)
        xn[:R, :C] = xc
        xn[:R, C] = 1.0
        xn_host = np.ascontiguousarray(
            xn.reshape(RC, 128, C + 1).transpose(1, 0, 2)).astype(BF)
        m = dict(shared)
        m["xT"] = xcT
        m["xN"] = xn_host
        in_maps.append(m)
    return in_maps


def kernel(x, Wqkv, g1, b1, Wproj, g2, b2, biases, bias_idxs):
    if "nc" not in _CACHE:
        _CACHE["nc"] = _build()
    nc = _CACHE["nc"]
    in_maps = _host_prep(x, Wqkv, g1, b1, Wproj, g2, b2, biases, bias_idxs)
    trace = bool(int(os.environ.get("BASS_ATT_TRACE", "0")))
    res = run_bass_kernel_spmd(nc, in_maps, list(range(NCORES)), trace=trace)
    _CACHE["last_result"] = res
    out = np.empty((B, N, C), np.float32)
    for c in range(NCORES):
        oc = res.results[c]["outT"]                        # [128, KC, R] f32
        oc = oc.transpose(1, 0, 2).reshape(C, R).T         # [R, C]
        out[NB * c: NB * (c + 1)] = oc.reshape(NB, N, C)
    return out


# revision 38
# speedup vs baseline: 1.0690x; 1.0690x over previous
"""LeViT-style attention block (qkv+BN -> biased softmax attention -> hardswish -> proj+BN)
on 8 Trainium2 NeuronCores, data-parallel over the batch dimension.

Self-contained: hardcodes shapes B=16, N=784, C=384, H=8.

Key device-side structure (per core, 2 batches):
- qkv matmul in bf16 (fp32 PSUM accumulate), channel slots padded to 64 per head.
  PSUM evacuation runs on the otherwise-idle ScalarE (ACT copy) so VectorE is
  free for the BN1 stats chain.
- BN1 statistics: Gram matrix G = [x|1]^T [x|1] on the PE concurrently with the
  qkv matmul, then H = W G locally and per-slot (sum y^2, sum y) packed into a
  12KB fp32 AllReduce (instead of AllReducing the 295KB Gram matrix).  1/sigma
  via exp(-0.5 ln(var+eps)) so the ACT table never swaps away from ln/exp.
- attention per (head-pair, batch): scores accumulate in PSUM, exp on ScalarE
  straight from PSUM, per-chunk multiply by the gathered exp(rel-pos bias) on
  VectorE/GpSimd, ones-row folded into V gives the softmax denominator,
  denominator reciprocal via exp(-ln d) on ScalarE + DMA partition-broadcast.
  The PE is power-throttled to ~50% duty after the first ~40us, so the design
  minimizes PE rows and keeps elementwise work off the PE.
- hardswish on VectorE; proj matmul + BN2 (bn_stats + small AllReduce).
"""
import os
import numpy as np
import ml_dtypes

import concourse.bass as bass
import concourse.mybir as mybir
import concourse.tile as tile
from concourse import bacc
from concourse.bass_utils import run_bass_kernel_spmd

F32 = mybir.dt.float32
BF16 = mybir.dt.bfloat16
BF = ml_dtypes.bfloat16

B, N, C, H = 16, 784, 384, 8
HD = 48            # head dim
HP = 64            # padded head dim (slot block size)
NCORES = 8
NB = B // NCORES   # batches per core = 2
R = NB * N         # rows per core = 1568
S1 = 3 * H * HP    # qkv slot count = 1536
MC1 = S1 // 128    # qkv m-chunks = 12
S2 = H * HP        # proj input slots = 512
KC = C // 128      # input channel chunks = 3
RC = 13            # row chunks for the Gram matmul (1568 -> 13*128 = 1664 padded)
EPS = 1e-5
SCALE = HD ** -0.5
NTOT = B * N       # total rows for BN stats = 12544
MCH = 7            # attention m-chunks (of 112) per 784
MCS = 112
# free-dim chunks that respect the 512-element PSUM bank boundary
NCH = [(0, 512), (512, 272)]
NCH_R = [(0, 512), (512, 512), (1024, 512), (1536, 32)]

_CACHE = {}

# Restrict the ACT table-set choices so exp, ln and copy share one set
# (natural_log_exp_and_others); the kernel only uses those so no
# ACT_TABLE_LOAD ever fires mid-kernel.
_orig_get_tables = bacc.get_activation_tables


def _patched_get_tables(arch):
    # Preserve entry order (act_func_set_id = index into act_info.json) but
    # empty the function lists of non-preferred sets so the chooser can't
    # pick them.
    t = _orig_get_tables(arch)
    keep = {"natural_log_exp_and_others"}
    return {k: (v if k in keep else set()) for k, v in t.items()}


bacc.get_activation_tables = _patched_get_tables


def _build():
    nc = bacc.Bacc("TRN2", target_bir_lowering=False, debug=False, num_devices=NCORES)

    xT_d = nc.dram_tensor("xT", [128, KC, R], BF16, kind="ExternalInput")
    xN_d = nc.dram_tensor("xN", [128, RC, C + 1], BF16, kind="ExternalInput")
    wq_d = nc.dram_tensor("wqT", [128, KC, S1], BF16, kind="ExternalInput")
    wqn_d = nc.dram_tensor("wqN", [128, MC1, C], BF16, kind="ExternalInput")
    wp_d = nc.dram_tensor("wpT", [128, S2 // 128, C], BF16, kind="ExternalInput")
    bias_d = nc.dram_tensor("biasg", [H, MCH, MCS, N], BF16, kind="ExternalInput")
    g1_d = nc.dram_tensor("g1p", [128, MC1], F32, kind="ExternalInput")
    b1_d = nc.dram_tensor("b1p", [128, MC1], F32, kind="ExternalInput")
    g2_d = nc.dram_tensor("g2p", [128, KC], F32, kind="ExternalInput")
    b2_d = nc.dram_tensor("b2p", [128, KC], F32, kind="ExternalInput")
    id_d = nc.dram_tensor("idm", [128, 128], BF16, kind="ExternalInput")
    ids_d = nc.dram_tensor("ids", [HP, HP], BF16, kind="ExternalInput")
    out_d = nc.dram_tensor("outT", [128, KC, R], BF16, kind="ExternalOutput")

    cc1_in = nc.dram_tensor("cc1_in", [128, MC1, 2], F32)
    cc1_out = nc.dram_tensor("cc1_out", [128, MC1, 2], F32, addr_space="Shared")
    rscr_d = nc.dram_tensor("rscr", [H // 2, NB, 2, N], F32)
    cc2_in = nc.dram_tensor("cc2_in", [128, KC, 2], F32)
    cc2_out = nc.dram_tensor("cc2_out", [128, KC, 2], F32, addr_space="Shared")

    AF = mybir.ActivationFunctionType
    OP = mybir.AluOpType
    PH = int(os.environ.get("BASS_ATT_PHASES", "4"))

    with tile.TileContext(nc) as tc:
        with tc.tile_pool(name="singles", bufs=1) as singles:
            xNs = singles.tile([128, RC, C + 1], BF16, tag="xN_s")
            nc.sync.dma_start(out=xNs[:], in_=xN_d.ap())
            wq = singles.tile([128, KC, S1], BF16)
            nc.sync.dma_start(out=wq[:], in_=wq_d.ap())
            wp = singles.tile([128, S2 // 128, C], BF16)
            nc.sync.dma_start(out=wp[:], in_=wp_d.ap())
            idm = singles.tile([128, 128], BF16)
            nc.sync.dma_start(out=idm[:], in_=id_d.ap())
            ids = singles.tile([HP, HP], BF16)
            nc.sync.dma_start(out=ids[:], in_=ids_d.ap())
            g1 = singles.tile([128, MC1], F32)
            nc.sync.dma_start(out=g1[:], in_=g1_d.ap())
            b1 = singles.tile([128, MC1], F32)
            nc.sync.dma_start(out=b1[:], in_=b1_d.ap())
            g2 = singles.tile([128, KC], F32)
            nc.sync.dma_start(out=g2[:], in_=g2_d.ap())
            b2 = singles.tile([128, KC], F32)
            nc.sync.dma_start(out=b2[:], in_=b2_d.ap())
            eps_t = singles.tile([128, 1], F32)
            nc.vector.memset(eps_t[:], EPS)

            y = singles.tile([128, MC1, R], BF16)            # qkv out, normalized in place
            o_pad = singles.tile([128, S2 // 128, R], BF16)  # attention output (padded slots)
            y2 = singles.tile([128, KC, R], F32)             # proj out, normalized in place
            scale1 = singles.tile([128, MC1], F32)
            shift1 = singles.tile([128, MC1], F32)
            lnd = singles.tile([33, N], F32)
            nc.vector.memset(lnd[:], 0.0)

            # ---------------- Phase 1: Gram stats (PE) + local H + small AllReduce, qkv matmul ----------------
            with tc.tile_pool(name="ph1", bufs=1) as ph1, \
                 tc.tile_pool(name="psg", bufs=2, space="PSUM") as psg, \
                 tc.tile_pool(name="psq", bufs=2, space="PSUM") as psq:
                xN = xNs
                xT = ph1.tile([128, KC, R], BF16)
                nc.sync.dma_start(out=xT[:], in_=xT_d.ap())
                wqn = ph1.tile([128, MC1, C], BF16)
                nc.sync.dma_start(out=wqn[:], in_=wqn_d.ap())
                gram_sb = ph1.tile([128, KC, C + 1], BF16)

                # Gram matrix G = [x|1]^T [x|1]  (col C holds the row-sums of x)
                for mc in range(KC):
                    gps = psg.tile([128, C + 1], F32, tag="gram")
                    for rc in range(RC):
                        nc.tensor.matmul(
                            gps[:],
                            xN[:, rc, mc * 128:(mc + 1) * 128],
                            xN[:, rc, :],
                            start=(rc == 0), stop=(rc == RC - 1),
                        )
                    nc.scalar.activation(gram_sb[:, mc, :], gps[:], AF.Copy)

                # H = W_pad @ G_local: per slot-chunk, H[:, :C] for sum(y^2),
                # H[:, C] = local sum(y).  Pack (sum y^2, sum y) per slot and
                # AllReduce just that (12KB fp32) instead of the Gram matrix.
                sumsq = ph1.tile([128, MC1], F32)
                sumY = ph1.tile([128, MC1], F32)
                scratch = ph1.tile([128, 2, C], F32)
                hsb = ph1.tile([128, MC1, C + 1], F32)
                for mc in range(MC1):
                    hps = psg.tile([128, C + 1], F32, tag="hmat")
                    for kc in range(KC):
                        nc.tensor.matmul(
                            hps[:],
                            wq[:, kc, mc * 128:(mc + 1) * 128],
                            gram_sb[:, kc, :],
                            start=(kc == 0), stop=(kc == KC - 1),
                        )
                    # fast PSUM evacuation on ScalarE so the slower VectorE
                    # reduction chain below doesn't pace the PE via the pool
                    nc.scalar.activation(hsb[:, mc, :], hps[:], AF.Copy)
                for mc in range(MC1):
                    # sumsq[slot] = sum_k W[slot,k] * H[slot,k]
                    sc = scratch[:, mc % 2, :]
                    nc.vector.tensor_tensor(sc, wqn[:, mc, :], hsb[:, mc, 0:C], op=OP.mult)
                    nc.vector.reduce_sum(sumsq[:, mc:mc + 1], sc, axis=mybir.AxisListType.X)
                    nc.vector.tensor_copy(sumY[:, mc:mc + 1], hsb[:, mc, C:C + 1])

                cc1_sb = ph1.tile([128, MC1, 2], F32)
                nc.vector.tensor_copy(cc1_sb[:, :, 0], sumsq[:])
                nc.vector.tensor_copy(cc1_sb[:, :, 1], sumY[:])
                cc1g = singles.tile([128, MC1, 2], F32)
                nc.gpsimd.dma_start(out=cc1_in.ap(), in_=cc1_sb[:])
                nc.gpsimd.collective_compute(
                    "AllReduce", OP.add,
                    ins=[cc1_in.ap()], outs=[cc1_out.ap()],
                    replica_groups=[list(range(NCORES))],
                )


                # qkv matmul (overlaps the Gram/AllReduce chain above); the
                # PSUM evacuation runs on ScalarE (ACT copy w/ cast) so the
                # VectorE stats chain above is never queued behind it.
                MC_ORDER = [0, 4, 8, 1, 5, 9, 2, 6, 10, 3, 7, 11]
                for mc in MC_ORDER:
                    for half in range(2):
                        ps = psq.tile([128, N], F32, tag="ps1")
                        h0 = half * N
                        for kc in range(KC):
                            for (off, sz) in NCH:
                                nc.tensor.matmul(
                                    ps[:, off:off + sz],
                                    wq[:, kc, mc * 128:(mc + 1) * 128],
                                    xT[:, kc, h0 + off:h0 + off + sz],
                                    start=(kc == 0), stop=(kc == KC - 1),
                                )
                        nc.scalar.activation(y[:, mc, h0:h0 + N], ps[:], AF.Copy)

            # ---------------- Phase 2a: v transposes on RAW v (no stats dep) ----------------
            # v_norm = a*v_raw + b and attention rows sum to 1 after the
            # denominator divide, so o = a*(attn@v_raw) + b: the BN affine is
            # applied per-partition in the tail instead of normalizing v.  The
            # transposes therefore run in the PE window that previously idled
            # waiting for the BN1 AllReduce (and mostly before the power
            # throttle kicks in); the ones-slot for the softmax denominator is
            # memset directly into the transposed tiles.
            v_exts = {}
            with tc.tile_pool(name="vop", bufs=2) as vop, \
                 tc.tile_pool(name="psvx", bufs=2, space="PSUM") as psvx:
                for h in range(H):
                    hp, eo = h // 2, h % 2
                    for b in range(NB):
                        col0 = b * N
                        # PE transposes need a base-0 source partition: odd
                        # head's v block staged down via DMA first
                        if eo == 0:
                            vs = lambda a, z: y[0:HP, 8 + hp, a:z]
                        else:
                            vodd = vop.tile([HP, N], BF16, tag="vodd")
                            nc.gpsimd.dma_start(
                                out=vodd[:], in_=y[HP:128, 8 + hp, col0:col0 + N])
                            vs = lambda a, z: vodd[:, a - col0:z - col0]
                        psv = psvx.tile([MCS, MCH, HP], BF16, tag="psv")
                        for mc in range(MCH):
                            nc.tensor.transpose(
                                psv[:, mc, :],
                                vs(col0 + mc * MCS, col0 + (mc + 1) * MCS),
                                ids[:],
                            )
                        vx = singles.tile([MCS, MCH, HP], BF16, tag=f"vx{h}_{b}")
                        # evacuate on ScalarE / memset on GpSimd: keeps these
                        # 32 ops off the VectorE queue so the AllReduce-gated
                        # normalize (and with it the first scores/exp) isn't
                        # serialized behind them
                        nc.scalar.activation(vx[:], psv[:], AF.Copy)
                        nc.gpsimd.memset(vx[:, :, 0], 1.0)
                        v_exts[(h, b)] = vx

            # scale/shift from global stats (attn scale folded into g1 on host).
            # This chain (and its ScalarE ln/exp, and the AllReduce-result DMA
            # on the VectorE queue) sits AFTER the raw-v work in program order
            # so no raw-y consumer inherits a semaphore target or in-order
            # queue position behind the AllReduce.
            nc.scalar.dma_start(out=cc1g[:], in_=cc1_out.ap())
            meanY = singles.tile([128, MC1], F32)
            varG = singles.tile([128, MC1], F32)
            nc.vector.tensor_scalar_mul(meanY[:], cc1g[:, :, 1], 1.0 / NTOT)
            nc.vector.tensor_scalar_mul(varG[:], cc1g[:, :, 0], 1.0 / NTOT)
            nc.vector.tensor_tensor(scale1[:], meanY[:], meanY[:], op=OP.mult)
            nc.vector.tensor_tensor(varG[:], varG[:], scale1[:], op=OP.subtract)
            # 1/sigma = exp(-0.5 * ln(var + eps)) -- stays in the ln/exp ACT table set
            nc.scalar.activation(varG[:], varG[:], AF.Ln, bias=eps_t[:])
            nc.scalar.activation(varG[:], varG[:], AF.Exp, scale=-0.5)
            nc.vector.tensor_tensor(scale1[:], g1[:], varG[:], op=OP.mult)
            nc.vector.tensor_tensor(shift1[:], meanY[:], scale1[:], op=OP.mult)
            nc.vector.tensor_tensor(shift1[:], b1[:], shift1[:], op=OP.subtract)

            # normalize q/k chunks in place (v stays raw), ordered so head 0's
            # chunks (q=0, k=4) finish first and attention can start early
            for mc in [0, 4, 1, 5, 2, 6, 3, 7]:
                nc.vector.tensor_scalar(
                    y[:, mc, :], y[:, mc, :],
                    scale1[:, mc:mc + 1], shift1[:, mc:mc + 1],
                    op0=OP.mult, op1=OP.add,
                )

            # ---------------- Phase 3: attention, one head per iteration ----------------
            # Software-pipelined: scores(mc+1) on the PE overlap exp(mc) on the
            # ScalarE (scores psum double-buffered); AV(mc-1) is emitted after
            # scores(mc) so the PE stream never stalls on the exp.
            if PH < 3:
                nc.vector.memset(o_pad[:], 0.0)
            if PH >= 3:
             with tc.tile_pool(name="biasp", bufs=2) as biasp, \
                  tc.tile_pool(name="ep", bufs=3) as ep, \
                  tc.tile_pool(name="tailp", bufs=2) as tailp, \
                  tc.tile_pool(name="pss", bufs=2, space="PSUM") as pss, \
                  tc.tile_pool(name="pso", bufs=2, space="PSUM") as pso:
                def emit_tail(hp, b, eo, lo, o_raw):
                    # r = exp(-ln(denom)); broadcast via DRAM DMA; normalize,
                    # v's BN affine (per-partition), and hardswish on VectorE
                    # into o_pad rows lo:lo+64
                    col0 = b * N
                    nc.scalar.activation(lnd[0:1, :], o_raw[0:1, :], AF.Ln)
                    r2 = tailp.tile([1, N], F32, tag="r2")
                    nc.scalar.activation(r2[:], lnd[0:1, :], AF.Exp, scale=-1.0)
                    rb = tailp.tile([HP, N], F32, tag="rb")
                    rsc = rscr_d.ap()[hp, b]
                    nc.gpsimd.dma_start(out=rsc[eo], in_=r2[0:1, :])
                    nc.gpsimd.dma_start(
                        out=rb[:],
                        in_=bass.AP(tensor=rsc.tensor, offset=rsc.offset + eo * N,
                                    ap=[[0, HP], [1, N]]),
                    )
                    oh = tailp.tile([HP, N], BF16, tag="oh")
                    nc.vector.tensor_tensor(oh[:], o_raw[:], rb[:], op=OP.mult)
                    oh2 = tailp.tile([HP, N], BF16, tag="oh2")
                    nc.vector.tensor_scalar(
                        oh2[:], oh[:],
                        scale1[lo:lo + HP, 8 + hp:8 + hp + 1],
                        shift1[lo:lo + HP, 8 + hp:8 + hp + 1],
                        op0=OP.mult, op1=OP.add,
                    )
                    t1 = tailp.tile([HP, N], BF16, tag="t1")
                    nc.vector.tensor_scalar(t1[:], oh2[:], 1.0 / 6.0, 0.5, op0=OP.mult, op1=OP.add)
                    t2 = tailp.tile([HP, N], BF16, tag="t2")
                    nc.vector.tensor_scalar(t2[:], t1[:], 1.0, 0.0, op0=OP.min, op1=OP.max)
                    nc.vector.tensor_tensor(
                        o_pad[lo:lo + HP, hp, col0:col0 + N], oh2[:], t2[:], op=OP.mult,
                    )

                pending_tail = None
                for h in range(H):
                    hp, eo = h // 2, h % 2
                    lo = eo * HP
                    bias_h = biasp.tile([MCS, MCH, N], BF16, tag="bias")
                    nc.sync.dma_start(
                        out=bias_h[:],
                        in_=bias_d.ap()[h].rearrange("m p n -> p m n"),
                    )
                    for b in range(NB):
                        col0 = b * N
                        # previous iteration's tail goes here so its Ln doesn't
                        # head-of-line block this iteration's exps on ScalarE
                        if pending_tail is not None:
                            emit_tail(*pending_tail)
                            pending_tail = None
                        v_ext = v_exts[(h, b)]

                        po = pso.tile([HP, N], F32, tag="po")
                        es = []
                        for mc in range(MCH):
                            m0 = col0 + mc * MCS
                            ps_s = pss.tile([MCS, N], F32, tag="se")
                            for (off, sz) in NCH:
                                nc.tensor.matmul(
                                    ps_s[:, off:off + sz],
                                    y[lo:lo + HD, 4 + hp, m0:m0 + MCS],
                                    y[lo:lo + HD, hp, col0 + off:col0 + off + sz],
                                    start=True, stop=True, skip_group_check=True,
                                )
                            # emit previous chunk's AV now so the PE stream has
                            # work while this chunk's exp runs on ScalarE
                            if mc >= 1:
                                e_prev, vslot = es[-1]
                                for (off, sz) in NCH:
                                    nc.tensor.matmul(
                                        po[:, off:off + sz], v_ext[:, vslot, :],
                                        e_prev[:, off:off + sz],
                                        start=(vslot == 0), stop=False, skip_group_check=True,
                                    )
                            e_c = ep.tile([MCS, N], BF16, tag="E")
                            nc.scalar.activation(e_c[:], ps_s[:], AF.Exp)
                            em = ep.tile([MCS, N], BF16, tag="Em")
                            nc.vector.tensor_tensor(em[:], e_c[:], bias_h[:, mc, :], op=OP.mult)
                            es.append((em, mc))
                        e_prev, vslot = es[-1]
                        for (off, sz) in NCH:
                            nc.tensor.matmul(
                                po[:, off:off + sz], v_ext[:, vslot, :],
                                e_prev[:, off:off + sz],
                                start=False, stop=True, skip_group_check=True,
                            )
                        # evacuate po early (frees the PSUM slot) and defer the
                        # rest of the tail into the next iteration
                        o_raw = tailp.tile([HP, N], F32, tag="oraw")
                        nc.vector.tensor_copy(o_raw[:], po[:])
                        pending_tail = (hp, b, eo, lo, o_raw)
                if pending_tail is not None:
                    emit_tail(*pending_tail)
                    pending_tail = None

            # ---------------- Phase 4: output projection + BN2 ----------------
            stats2 = singles.tile([128, KC, 4, 6], F32)
            mv2 = singles.tile([128, KC, 2], F32)
            cc2_sb = singles.tile([128, KC, 2], F32)
            cc2g = singles.tile([128, KC, 2], F32)
            with tc.tile_pool(name="psq2", bufs=2, space="PSUM") as psq2:
                for mc in range(KC):
                    ps = psq2.tile([128, R], F32, tag="ps2")
                    for kc in range(S2 // 128):
                        for (off, sz) in NCH_R:
                            nc.tensor.matmul(
                                ps[:, off:off + sz],
                                wp[:, kc, mc * 128:(mc + 1) * 128],
                                o_pad[:, kc, off:off + sz],
                                start=(kc == 0), stop=(kc == S2 // 128 - 1),
                            )
                    # evacuate on ScalarE (idle after the last exp) so VectorE
                    # can start bn_stats the moment each chunk lands
                    nc.scalar.activation(y2[:, mc, :], ps[:], AF.Copy)
                    for c4 in range(4):
                        nc.vector.bn_stats(stats2[:, mc, c4, :], y2[:, mc, c4 * 392:(c4 + 1) * 392])
                    nc.vector.bn_aggr(mv2[:, mc, :], stats2[:, mc, :, :])

            nc.vector.tensor_scalar_mul(cc2_sb[:, :, 0], mv2[:, :, 0], float(R))
            nc.vector.tensor_tensor(cc2_sb[:, :, 1], mv2[:, :, 0], mv2[:, :, 0], op=OP.mult)
            nc.vector.tensor_tensor(cc2_sb[:, :, 1], cc2_sb[:, :, 1], mv2[:, :, 1], op=OP.add)
            nc.vector.tensor_scalar_mul(cc2_sb[:, :, 1], cc2_sb[:, :, 1], float(R))
            nc.gpsimd.dma_start(out=cc2_in.ap(), in_=cc2_sb[:])
            nc.gpsimd.collective_compute(
                "AllReduce", OP.add,
                ins=[cc2_in.ap()], outs=[cc2_out.ap()],
                replica_groups=[list(range(NCORES))],
            )
            nc.gpsimd.dma_start(out=cc2g[:], in_=cc2_out.ap())

            mean2 = singles.tile([128, KC], F32)
            var2 = singles.tile([128, KC], F32)
            scale2 = singles.tile([128, KC], F32)
            shift2 = singles.tile([128, KC], F32)
            nc.vector.tensor_scalar_mul(mean2[:], cc2g[:, :, 0], 1.0 / NTOT)
            nc.vector.tensor_scalar_mul(var2[:], cc2g[:, :, 1], 1.0 / NTOT)
            nc.vector.tensor_tensor(scale2[:], mean2[:], mean2[:], op=OP.mult)
            nc.vector.tensor_tensor(var2[:], var2[:], scale2[:], op=OP.subtract)
            nc.scalar.activation(var2[:], var2[:], AF.Ln, bias=eps_t[:])
            nc.scalar.activation(var2[:], var2[:], AF.Exp, scale=-0.5)
            nc.vector.tensor_tensor(scale2[:], g2[:], var2[:], op=OP.mult)
            nc.vector.tensor_tensor(shift2[:], mean2[:], scale2[:], op=OP.mult)
            nc.vector.tensor_tensor(shift2[:], b2[:], shift2[:], op=OP.subtract)
            # normalize into the (dead) y tile as bf16: 2x DVE write rate and
            # half the HBM-out bytes vs fp32-in-place; host casts back to f32
            for mc in range(KC):
                nc.vector.tensor_scalar(
                    y[:, mc, :], y2[:, mc, :],
                    scale2[:, mc:mc + 1], shift2[:, mc:mc + 1],
                    op0=OP.mult, op1=OP.add,
                )
                nc.sync.dma_start(out=out_d.ap()[:, mc], in_=y[:, mc, :])

    nc.compile()
    return nc


def _host_prep(x, Wqkv, g1, b1, Wproj, g2, b2, biases, bias_idxs):
    x = np.asarray(x, dtype=np.float32)
    Wqkv = np.asarray(Wqkv, dtype=np.float32)
    g1 = np.asarray(g1, dtype=np.float32)
    b1 = np.asarray(b1, dtype=np.float32)
    Wproj = np.asarray(Wproj, dtype=np.float32)
    g2 = np.asarray(g2, dtype=np.float32)
    b2 = np.asarray(b2, dtype=np.float32)
    biases = np.asarray(biases, dtype=np.float32)
    bias_idxs = np.asarray(bias_idxs)

    # channel slot layout: block = t*8+h (t in q,k,v), 64 slots per block.
    # q/k blocks: dims at slots 0..47.  v blocks: ones-row at slot 0 (32-aligned
    # partition for the denominator), dims at slots 1..48.
    g1e = g1.copy()
    b1e = b1.copy()
    g1e[:C] *= SCALE   # fold attention scale into q channels
    b1e[:C] *= SCALE
    Wq_pad = np.zeros((S1, C), np.float32)
    g1_pad = np.zeros(S1, np.float32)
    b1_pad = np.zeros(S1, np.float32)
    for t in range(3):
        for h in range(H):
            blk = (t * H + h) * HP
            d0 = blk + (1 if t == 2 else 0)
            Wq_pad[d0:d0 + HD] = Wqkv[t * C + h * HD: t * C + h * HD + HD]
            g1_pad[d0:d0 + HD] = g1e[t * C + h * HD: t * C + h * HD + HD]
            b1_pad[d0:d0 + HD] = b1e[t * C + h * HD: t * C + h * HD + HD]
    for h in range(H):
        b1_pad[(2 * H + h) * HP] = 1.0

    wq_host = np.ascontiguousarray(
        Wq_pad.T.reshape(KC, 128, S1).transpose(1, 0, 2)).astype(BF)
    wqn_host = np.ascontiguousarray(
        Wq_pad.reshape(MC1, 128, C).transpose(1, 0, 2)).astype(BF)
    g1_host = np.ascontiguousarray(g1_pad.reshape(MC1, 128).T)
    b1_host = np.ascontiguousarray(b1_pad.reshape(MC1, 128).T)

    Wp_pad = np.zeros((S2, C), np.float32)   # [slot_in, c_out]; slot 0 (denom) stays zero
    for h in range(H):
        Wp_pad[h * HP + 1:h * HP + 1 + HD] = Wproj[:, h * HD:h * HD + HD].T
    wp_host = np.ascontiguousarray(
        Wp_pad.reshape(S2 // 128, 128, C).transpose(1, 0, 2)).astype(BF)
    g2_host = np.ascontiguousarray(g2.reshape(KC, 128).T)
    b2_host = np.ascontiguousarray(b2.reshape(KC, 128).T)

    bias_gather = biases[:, bias_idxs]                     # [H, N, N] fp32
    bias_host = np.ascontiguousarray(
        np.exp(bias_gather).reshape(H, MCH, MCS, N)).astype(BF)

    id_host = np.zeros((128, 128), np.float32)
    id_host[0:112, 0:112] = np.eye(112)
    id_host = id_host.astype(BF)
    ids_host = np.eye(HP, dtype=np.float32).astype(BF)

    shared = {
        "wqT": wq_host, "wqN": wqn_host, "wpT": wp_host, "biasg": bias_host,
        "g1p": g1_host, "b1p": b1_host, "g2p": g2_host, "b2p": b2_host,
        "idm": id_host, "ids": ids_host,
    }
    in_maps = []
    for c in range(NCORES):
        xc = x[NB * c: NB * (c + 1)].reshape(R, C)         # [R, C]
        xcT = np.ascontiguousarray(
            xc.T.reshape(KC, 128, R).transpose(1, 0, 2)).astype(BF)
        # natural layout rows padded to RC*128 with an appended ones column
        xn = np.zeros((RC * 128, C + 1), np.float32)
        xn[:R, :C] = xc
        xn[:R, C] = 1.0
        xn_host = np.ascontiguousarray(
            xn.reshape(RC, 128, C + 1).transpose(1, 0, 2)).astype(BF)
        m = dict(shared)
        m["xT"] = xcT
        m["xN"] = xn_host
        in_maps.append(m)
    return in_maps


def kernel(x, Wqkv, g1, b1, Wproj, g2, b2, biases, bias_idxs):
    if "nc" not in _CACHE:
        _CACHE["nc"] = _build()
    nc = _CACHE["nc"]
    in_maps = _host_prep(x, Wqkv, g1, b1, Wproj, g2, b2, biases, bias_idxs)
    trace = bool(int(os.environ.get("BASS_ATT_TRACE", "0")))
    res = run_bass_kernel_spmd(nc, in_maps, list(range(NCORES)), trace=trace)
    _CACHE["last_result"] = res
    out = np.empty((B, N, C), np.float32)
    for c in range(NCORES):
        oc = res.results[c]["outT"].astype(np.float32)     # [128, KC, R] bf16 -> f32
        oc = oc.transpose(1, 0, 2).reshape(C, R).T         # [R, C]
        out[NB * c: NB * (c + 1)] = oc.reshape(NB, N, C)
    return out
